# revision 25
# baseline (speedup 1.0000x reference)
"""DiffConv (graph diffusion convolution) Trainium2 kernel — fp8 DoubleRow.

Math (reference):
    out = sum_{k=0..2} A^k @ (H @ Wf[k]) + (A^T)^k @ (H @ Wb[k]) + bias
with H [b=8, t=24, n=1024, d=64], A [t, n, n], Wf/Wb [3, d, d].

Factorization (per t, batches packed into the matmul free dim):
    U0 = H @ (Wf0 + Wb0) + bias      (HOST, exact fp32 -> bf16; dominates
                                      the output, A-chain terms ~70x smaller)
    U1 = H@Wf1, U2 = H@Wf2, V2 = H@Wb2, V1 = H@Wb1   (on-chip "W-phase", bf16)
    out = U0 + A @ (U1 + A @ U2) + A^T @ (V1 + A^T @ V2)     (Horner)

Key speed tricks vs the fp32r version (327 us -> 143 us measured):
  * All four spmms run as fp8e4 DoubleRow matmuls: A is pre-scaled by 2^13
    on the host (A in [0, 1/N] underflows e4m3 otherwise) and stored fp8;
    U1/U2/V2/V1/S_f/S_b live in fp8.  DoubleRow contracts TWO 128-row
    blocks per matmul (lhsT/rhs get [128, 2, cols] APs), halving PE
    cycles vs fp32r/bf16.  Evictions rescale by 2^-13 via fused
    scalar_tensor_tensor.  The 2e-2 rel-err gate tolerates this easily
    (fp8 errors average down ~sqrt(1024) in the contraction; the A-chain
    terms are ~70x below the output scale; measured rel err 7e-3).
  * A / A^T HBM traffic drops 4x (fp8), H / U0 / out 2x (bf16).
  * F and B spmms accumulate into ONE psum group -> single eviction.
  * Eviction work is split DVE/ACT (5:3 toward ACT for the W-phase casts
    interleaved with FB, where DVE also runs the spmm STTs).  GpSimd has
    no PSUM port on TRN2.  NOTE: the two 64-row half-array W matmuls must
    write SEPARATE psum banks — sharing one 2KB bank as a single
    accumulation group crashes the exec unit (NRT_EXEC_UNIT_UNRECOVERABLE).
  * t=0's U1/U2/V2/V1 are host-precomputed (UV0P, slot-major so the DMA
    is contiguous) so no W(0) phase delays the pipeline start; ~3.4us of
    dummy matmuls warm the HAM clock gate while the prologue DMAs land.
  * Per-n-block output stores overlap the FB phase (no store tail).

Sharding: t (24 steps) across 8 cores (3 each), zero collectives; A is
read exactly once fleet-wide.  All device tensors host-pre-permuted to
SBUF-native layouts; every DMA is a large contiguous transfer.
"""

import os
import sys

sys.path.insert(0, "/opt/trn_rl_repo")

import numpy as np
import ml_dtypes

PHASES = os.environ.get("DIFFCONV_PHASES", "wfbo")  # debug: subset of "wfbo"

import concourse.tile as tile
from concourse import bacc, mybir
from concourse.bass_utils import run_bass_kernel_spmd

B, T, N, D = 8, 24, 1024, 64
NCORES = 8
TPC = T // NCORES  # t-steps per core
NB = N // 128  # 128-row blocks of n
BD = B * D
F32 = mybir.dt.float32
BF16 = mybir.dt.bfloat16
F8 = mybir.dt.float8e4
DR = mybir.MatmulPerfMode.DoubleRow
MUL = mybir.AluOpType.mult
ADD = mybir.AluOpType.add
ASCALE = 2.0**13  # host premultiplier keeping fp8 A out of subnormals
SCL = float(2.0**-13)

_cached = {}


def _build():
    if "nc" in _cached:
        return _cached["nc"]

    nc = bacc.Bacc("TRN2", target_bir_lowering=False, debug=False)
    dHT = nc.dram_tensor("HTP", [TPC, 128, 4, N], BF16, kind="ExternalInput")
    dAT = nc.dram_tensor("ATP", [TPC, 128, NB, N], F8, kind="ExternalInput")
    dA = nc.dram_tensor("AP", [TPC, 128, NB, N], F8, kind="ExternalInput")
    dW = nc.dram_tensor("Wcat", [D, 4 * D], BF16, kind="ExternalInput")
    dU0 = nc.dram_tensor("U0P", [TPC, 128, NB, BD], BF16, kind="ExternalInput")
    dUV0 = nc.dram_tensor("UV0P", [128, 4, NB, BD], F8, kind="ExternalInput")
    dOUT = nc.dram_tensor("out", [TPC, 128, NB, BD], BF16, kind="ExternalOutput")

    with tile.TileContext(nc) as tc:
        with (
            tc.tile_pool(name="wc", bufs=1) as wpool,
            tc.tile_pool(name="amat", bufs=4) as apool,
            tc.tile_pool(name="ht", bufs=2) as hpool,
            tc.tile_pool(name="uv4", bufs=2) as uvpool,
            tc.tile_pool(name="osb", bufs=2) as opool,
            tc.tile_pool(name="wps", bufs=5, space="PSUM") as wps,
            tc.tile_pool(name="sps", bufs=3, space="PSUM") as sps,
        ):
            # Wcat [64, 256] = [Wf1|Wf2|Wb2|Wb1], replicated on both
            # partition halves (W-phase runs two batches concurrently on
            # the two 64-row halves of the PE array).
            wc = wpool.tile([128, 4 * D], BF16)
            # HAM warmup: ~7us of N=512 dummy matmuls on a zeroed scratch
            # while the prologue DMAs stream in, so the real matmuls start
            # at 2.4 GHz with no MID-window re-throttle.
            warm = opool.tile([128, NB, BD], BF16, tag="osb")
            nc.vector.memset(warm[:, 0], 0.0)
            for wi in range(32):
                wps_w = wps.tile([128, 2, 4 * D], F32, tag="wps")
                nc.tensor.matmul(
                    wps_w[:].rearrange("p a b -> p (a b)"),
                    warm[:, 0, 0:128],
                    warm[:, 0],
                    start=True,
                    stop=True,
                )

            hts = {}
            uv4s = {}

            def load_ht(t):
                # partition = (b%2)*64 + d, free = (b//2, n)
                ht = hpool.tile([128, 4, N], BF16, tag="ht")
                nc.gpsimd.dma_start(ht[:], dHT.ap()[t])
                hts[t] = ht

            def alloc_uv(t):
                # uv4 slots: 0=U1 (becomes S_f), 1=U2, 2=V2, 3=V1 (becomes S_b)
                uv4s[t] = uvpool.tile(
                    [128, 4, NB, BD], F8, tag="uv4", name=f"uv4_{t}"
                )

            def w_step(t, nb, bp, act_heavy=True):
                """W-phase unit: one b-pair (b=2bp, 2bp+1) x one n-block.

                Two half-array matmuls (K=64 each, concurrent on the two
                row halves via auto tile_position) into separate psum
                banks; each evicted by one strided fp8 cast.
                """
                ht, uv4 = hts[t], uv4s[t]
                for b2 in range(2):
                    b = 2 * bp + b2
                    ps = wps.tile([128, 4 * D], F32, tag="wps")
                    nc.tensor.matmul(
                        ps[:],
                        ht[b2 * 64 : b2 * 64 + 64, bp, nb * 128 : (nb + 1) * 128],
                        wc[b2 * 64 : b2 * 64 + 64, :],
                        start=True,
                        stop=True,
                    )
                    src = ps[:].rearrange("p (w d) -> p w d", w=4)
                    dst = uv4[:, :, nb, b * 64 : (b + 1) * 64]
                    # DVE also runs the spmm STT evictions in steady state;
                    # keep it at 3 of 8 W-casts there (5:3 ACT:DVE).
                    on_dve = (bp * 2 + b2 >= 5) if act_heavy else (b2 == 0)
                    if on_dve:
                        nc.vector.tensor_copy(dst, src)
                    else:
                        nc.scalar.copy(dst, src)

            def dr_mms(ps, stat, i, slot_of, start, stop):
                """4 DoubleRow matmuls: out_block i, contraction over the
                4 pairs of 128-row n-blocks."""
                for jp in range(NB // 2):
                    nc.tensor.matmul(
                        ps[:],
                        stat[:, 2 * jp : 2 * jp + 2, i * 128 : (i + 1) * 128],
                        slot_of(jp),
                        start=(start and jp == 0),
                        stop=(stop and jp == NB // 2 - 1),
                        perf_mode=DR,
                    )

            # ---------------- t = 0 prologue ----------------
            # t=0's U1/U2/V2/V1 come host-precomputed (dUV0) so no W(0)
            # phase blocks the pipeline start; slots 0-1 load first so T_f
            # can begin while slots 2-3 stream in.
            at = apool.tile([128, NB, N], F8, tag="am")
            nc.gpsimd.dma_start(at[:, 0:4], dAT.ap()[0][:, 0:4])
            am = apool.tile([128, NB, N], F8, tag="am")
            nc.gpsimd.dma_start(am[:, 0:4], dA.ap()[0][:, 0:4])
            nc.gpsimd.dma_start(at[:, 4:8], dAT.ap()[0][:, 4:8])
            nc.gpsimd.dma_start(am[:, 4:8], dA.ap()[0][:, 4:8])
            alloc_uv(0)
            nc.sync.dma_start(uv4s[0][:, 1], dUV0.ap()[:, 1])
            nc.sync.dma_start(uv4s[0][:, 0], dUV0.ap()[:, 0])
            nc.sync.dma_start(uv4s[0][:, 2:4], dUV0.ap()[:, 2:4])
            nc.gpsimd.dma_start(wc[0:64, :], dW.ap())
            nc.gpsimd.dma_start(wc[64:128, :], dW.ap())

            for t in range(TPC):
                uv4 = uv4s[t]
                if t + 1 < TPC:
                    load_ht(t + 1)  # W(t+1) reads it during FB(t)
                osb = opool.tile([128, NB, BD], BF16, tag="osb")

                # ---- T_f: S_f = U1 + 2^-13 * Ahat @ U2 (slot0 in place) ----
                for i in range(NB if "f" in PHASES else 0):
                    ps = sps.tile([128, BD], F32)
                    dr_mms(ps, at, i, lambda jp: uv4[:, 1, 2 * jp : 2 * jp + 2],
                           start=True, stop=True)
                    nc.vector.scalar_tensor_tensor(
                        uv4[:, 0, i], ps[:], SCL, uv4[:, 0, i], op0=MUL, op1=ADD
                    )
                nc.sync.dma_start(osb[:], dU0.ap()[t])  # needed at FB STTs
                # ---- T_b: S_b = V1 + 2^-13 * Abar @ V2 (slot3 in place) ----
                for i in range(NB if "b" in PHASES else 0):
                    ps = sps.tile([128, BD], F32)
                    dr_mms(ps, am, i, lambda jp: uv4[:, 2, 2 * jp : 2 * jp + 2],
                           start=True, stop=True)
                    nc.vector.scalar_tensor_tensor(
                        uv4[:, 3, i], ps[:], SCL, uv4[:, 3, i], op0=MUL, op1=ADD
                    )
                # ---- FB: osb += 2^-13 * (Ahat @ S_f + Abar @ S_b); W(t+1) ----
                if t + 1 < TPC:
                    at_n = apool.tile([128, NB, N], F8, tag="am")
                    nc.gpsimd.dma_start(at_n[:], dAT.ap()[t + 1])
                    am_n = apool.tile([128, NB, N], F8, tag="am")
                    nc.gpsimd.dma_start(am_n[:], dA.ap()[t + 1])
                    alloc_uv(t + 1)
                for i in range(NB if "o" in PHASES else 0):
                    ps = sps.tile([128, BD], F32)
                    dr_mms(ps, at, i, lambda jp: uv4[:, 0, 2 * jp : 2 * jp + 2],
                           start=True, stop=False)
                    dr_mms(ps, am, i, lambda jp: uv4[:, 3, 2 * jp : 2 * jp + 2],
                           start=False, stop=True)
                    nc.vector.scalar_tensor_tensor(
                        osb[:, i], ps[:], SCL, osb[:, i], op0=MUL, op1=ADD
                    )
                    nc.sync.dma_start(dOUT.ap()[t][:, i], osb[:, i])
                    if t + 1 < TPC and "w" in PHASES:
                        # pipelined W-phase keeps the PE dense
                        for bp in range(4):
                            w_step(t + 1, i, bp)

                if "o" not in PHASES:
                    if "w" in PHASES and t + 1 < TPC:
                        for i in range(NB):
                            for bp in range(4):
                                w_step(t + 1, i, bp)
                    nc.sync.dma_start(dOUT.ap()[t], osb[:])
                if t + 1 < TPC:
                    at, am = at_n, am_n

    nc.compile()
    _cached["nc"] = nc
    return nc


def _prep_core(H, A, AT, U0, Wcat, c):
    WUV = _cached["WUV"]
    ts = slice(c * TPC, (c + 1) * TPC)
    # HTP[t, (b%2)*64+d, b//2, n] = H[b, t, n, d]
    Ht = H[:, ts]  # [8, TPC, N, D]
    HTP = (
        Ht.transpose(1, 0, 3, 2)  # [t, b, d, n]
        .reshape(TPC, 4, 2, D, N)  # b = b1*2 + b2
        .transpose(0, 2, 3, 1, 4)  # [t, b2, d, b1, n]
        .reshape(TPC, 128, 4, N)
    )
    # A/AT: [t, p, j, c] with row = j*128+p; scaled into fp8 range
    APc = A[ts].reshape(TPC, NB, 128, N).transpose(0, 2, 1, 3) * ASCALE
    ATPc = AT[ts].reshape(TPC, NB, 128, N).transpose(0, 2, 1, 3) * ASCALE
    # U0P[t, p, i, b*64+d] = U0[b, t, i*128+p, d]
    U0P = (
        U0[:, ts]  # [b, t, n, d]
        .transpose(1, 2, 0, 3)  # [t, n, b, d]
        .reshape(TPC, NB, 128, B, D)
        .transpose(0, 2, 1, 3, 4)  # [t, p, i, b, d]
        .reshape(TPC, 128, NB, BD)
    )
    # UV0P[p, i, s, b*64+d] = [U1,U2,V2,V1][s][b, t0, i*128+p, d] for t0 = first t
    t0 = c * TPC
    X = np.stack([H[:, t0] @ W1 for W1 in WUV], axis=0)  # [4, B, N, D]
    UV0P = (
        X.transpose(2, 0, 1, 3)  # [N, 4, B, D]
        .reshape(NB, 128, 4, B * D)
        .transpose(1, 2, 0, 3)  # [p, s, i, bd]
    )
    return {
        "UV0P": np.ascontiguousarray(UV0P.astype(ml_dtypes.float8_e4m3)),
        "HTP": np.ascontiguousarray(HTP).astype(ml_dtypes.bfloat16),
        "ATP": np.ascontiguousarray(ATPc.astype(ml_dtypes.float8_e4m3)),
        "AP": np.ascontiguousarray(APc.astype(ml_dtypes.float8_e4m3)),
        "Wcat": Wcat,
        "U0P": np.ascontiguousarray(U0P).astype(ml_dtypes.bfloat16),
    }


def kernel(H, A, Wf, Wb, bias):
    H = np.ascontiguousarray(np.asarray(H, dtype=np.float32))
    A = np.ascontiguousarray(np.asarray(A, dtype=np.float32))
    Wf = np.asarray(Wf, dtype=np.float32)
    Wb = np.asarray(Wb, dtype=np.float32)
    bias = np.asarray(bias, dtype=np.float32)

    AT = np.ascontiguousarray(A.transpose(0, 2, 1))
    U0 = (H @ (Wf[0] + Wb[0]) + bias).astype(np.float32)
    # w-slot order U1, U2, V2, V1
    Wcat = np.ascontiguousarray(
        np.concatenate([Wf[1], Wf[2], Wb[2], Wb[1]], axis=1)
    ).astype(ml_dtypes.bfloat16)

    _cached["WUV"] = [Wf[1], Wf[2], Wb[2], Wb[1]]
    nc = _build()
    in_maps = [_prep_core(H, A, AT, U0, Wcat, c) for c in range(NCORES)]
    res = run_bass_kernel_spmd(nc, in_maps, core_ids=list(range(NCORES)))

    # out dram is [t, p, i, (b d)] kernel-native bf16; un-permute on host.
    outp = np.concatenate(
        [res.results[c]["out"].astype(np.float32) for c in range(NCORES)], axis=0
    )
    out = (
        outp.reshape(T, 128, NB, B, D)
        .transpose(3, 0, 2, 1, 4)  # [b, t, i, p, d]
        .reshape(B, T, N, D)
    )
    return np.ascontiguousarray(out)


# revision 26
# speedup vs baseline: 1.0437x; 1.0437x over previous
"""DiffConv (graph diffusion convolution) Trainium2 kernel — fp8 DoubleRow.

Math (reference):
    out = sum_{k=0..2} A^k @ (H @ Wf[k]) + (A^T)^k @ (H @ Wb[k]) + bias
with H [b=8, t=24, n=1024, d=64], A [t, n, n], Wf/Wb [3, d, d].

Factorization (per t, batches packed into the matmul free dim):
    U0 = H @ (Wf0 + Wb0) + bias      (HOST, exact fp32 -> bf16; dominates
                                      the output, A-chain terms ~70x smaller)
    U1 = H@Wf1, U2 = H@Wf2, V2 = H@Wb2, V1 = H@Wb1   (on-chip "W-phase", bf16)
    out = U0 + A @ (U1 + A @ U2) + A^T @ (V1 + A^T @ V2)     (Horner)

Key speed tricks vs the fp32r version (327 us -> 143 us measured):
  * All four spmms run as fp8e4 DoubleRow matmuls: A is pre-scaled by 2^13
    on the host (A in [0, 1/N] underflows e4m3 otherwise) and stored fp8;
    U1/U2/V2/V1/S_f/S_b live in fp8.  DoubleRow contracts TWO 128-row
    blocks per matmul (lhsT/rhs get [128, 2, cols] APs), halving PE
    cycles vs fp32r/bf16.  Evictions rescale by 2^-13 via fused
    scalar_tensor_tensor.  The 2e-2 rel-err gate tolerates this easily
    (fp8 errors average down ~sqrt(1024) in the contraction; the A-chain
    terms are ~70x below the output scale; measured rel err 7e-3).
  * A / A^T HBM traffic drops 4x (fp8), H / U0 / out 2x (bf16).
  * F and B spmms accumulate into ONE psum group -> single eviction.
  * Eviction work is split DVE/ACT (5:3 toward ACT for the W-phase casts
    interleaved with FB, where DVE also runs the spmm STTs).  GpSimd has
    no PSUM port on TRN2.  NOTE: the two 64-row half-array W matmuls must
    write SEPARATE psum banks — sharing one 2KB bank as a single
    accumulation group crashes the exec unit (NRT_EXEC_UNIT_UNRECOVERABLE).
  * t=0's U1/U2/V2/V1 are host-precomputed (UV0P, slot-major so the DMA
    is contiguous) so no W(0) phase delays the pipeline start; ~3.4us of
    dummy matmuls warm the HAM clock gate while the prologue DMAs land.
  * Per-n-block output stores overlap the FB phase (no store tail).

Sharding: t (24 steps) across 8 cores (3 each), zero collectives; A is
read exactly once fleet-wide.  All device tensors host-pre-permuted to
SBUF-native layouts; every DMA is a large contiguous transfer.
"""

import os
import sys

sys.path.insert(0, "/opt/trn_rl_repo")

import numpy as np
import ml_dtypes

PHASES = os.environ.get("DIFFCONV_PHASES", "wfbo")  # debug: subset of "wfbo"

import concourse.tile as tile
from concourse import bacc, mybir
from concourse.bass_utils import run_bass_kernel_spmd

B, T, N, D = 8, 24, 1024, 64
NCORES = 8
TPC = T // NCORES  # t-steps per core
NB = N // 128  # 128-row blocks of n
BD = B * D
F32 = mybir.dt.float32
BF16 = mybir.dt.bfloat16
F8 = mybir.dt.float8e4
DR = mybir.MatmulPerfMode.DoubleRow
MUL = mybir.AluOpType.mult
ADD = mybir.AluOpType.add
ASCALE = 2.0**13  # host premultiplier keeping fp8 A out of subnormals
SCL = float(2.0**-13)

_cached = {}


def _build():
    if "nc" in _cached:
        return _cached["nc"]

    nc = bacc.Bacc("TRN2", target_bir_lowering=False, debug=False)
    dHT = nc.dram_tensor("HTP", [TPC, 128, 4, N], BF16, kind="ExternalInput")
    dAT = nc.dram_tensor("ATP", [TPC, 128, NB, N], F8, kind="ExternalInput")
    dA = nc.dram_tensor("AP", [TPC, 128, NB, N], F8, kind="ExternalInput")
    dW = nc.dram_tensor("Wcat", [D, 4 * D], BF16, kind="ExternalInput")
    dU0 = nc.dram_tensor("U0P", [TPC, 128, NB, BD], BF16, kind="ExternalInput")
    dUV0 = nc.dram_tensor("UV0P", [128, 4, NB, BD], F8, kind="ExternalInput")
    dOUT = nc.dram_tensor("out", [TPC, 128, NB, BD], BF16, kind="ExternalOutput")

    with tile.TileContext(nc) as tc:
        with (
            tc.tile_pool(name="wc", bufs=1) as wpool,
            tc.tile_pool(name="amat", bufs=4) as apool,
            tc.tile_pool(name="ht", bufs=2) as hpool,
            tc.tile_pool(name="uv4", bufs=2) as uvpool,
            tc.tile_pool(name="osb", bufs=2) as opool,
            tc.tile_pool(name="wps", bufs=5, space="PSUM") as wps,
            tc.tile_pool(name="sps", bufs=3, space="PSUM") as sps,
        ):
            # Wcat [64, 256] = [Wf1|Wf2|Wb2|Wb1], replicated on both
            # partition halves (W-phase runs two batches concurrently on
            # the two 64-row halves of the PE array).
            wc = wpool.tile([128, 4 * D], BF16)
            # HAM warmup: ~3.4us of dummy matmuls on zeroed wc while the
            # prologue DMAs stream in, so real matmuls start at 2.4 GHz.
            nc.vector.memset(wc[:], 0.0)
            for wi in range(32):
                wps_w = wps.tile([128, 4 * D], F32, tag="wps")
                nc.tensor.matmul(
                    wps_w[:], wc[:, 0:128], wc[:], start=True, stop=True
                )

            hts = {}
            uv4s = {}

            def load_ht(t):
                # partition = (b%2)*64 + d, free = (b//2, n)
                ht = hpool.tile([128, 4, N], BF16, tag="ht")
                nc.gpsimd.dma_start(ht[:], dHT.ap()[t])
                hts[t] = ht

            def alloc_uv(t):
                # uv4 slots: 0=U1 (becomes S_f), 1=U2, 2=V2, 3=V1 (becomes S_b)
                uv4s[t] = uvpool.tile(
                    [128, 4, NB, BD], F8, tag="uv4", name=f"uv4_{t}"
                )

            def w_step(t, nb, bp, act_heavy=True):
                """W-phase unit: one b-pair (b=2bp, 2bp+1) x one n-block.

                Two half-array matmuls (K=64 each, concurrent on the two
                row halves via auto tile_position) into separate psum
                banks; each evicted by one strided fp8 cast.
                """
                ht, uv4 = hts[t], uv4s[t]
                for b2 in range(2):
                    b = 2 * bp + b2
                    ps = wps.tile([128, 4 * D], F32, tag="wps")
                    nc.tensor.matmul(
                        ps[:],
                        ht[b2 * 64 : b2 * 64 + 64, bp, nb * 128 : (nb + 1) * 128],
                        wc[b2 * 64 : b2 * 64 + 64, :],
                        start=True,
                        stop=True,
                    )
                    src = ps[:].rearrange("p (w d) -> p w d", w=4)
                    dst = uv4[:, :, nb, b * 64 : (b + 1) * 64]
                    # DVE also runs the spmm STT evictions in steady state;
                    # keep it at 3 of 8 W-casts there (5:3 ACT:DVE).
                    on_dve = (bp * 2 + b2 >= 5) if act_heavy else (b2 == 0)
                    if on_dve:
                        nc.vector.tensor_copy(dst, src)
                    else:
                        nc.scalar.copy(dst, src)

            def dr_mms(ps, stat, i, slot_of, start, stop):
                """4 DoubleRow matmuls: out_block i, contraction over the
                4 pairs of 128-row n-blocks."""
                for jp in range(NB // 2):
                    nc.tensor.matmul(
                        ps[:],
                        stat[:, 2 * jp : 2 * jp + 2, i * 128 : (i + 1) * 128],
                        slot_of(jp),
                        start=(start and jp == 0),
                        stop=(stop and jp == NB // 2 - 1),
                        perf_mode=DR,
                    )

            # ---------------- t = 0 prologue ----------------
            # t=0's U1/U2/V2/V1 come host-precomputed (dUV0) so no W(0)
            # phase blocks the pipeline start; slots 0-1 load first so T_f
            # can begin while slots 2-3 stream in.
            at = apool.tile([128, NB, N], F8, tag="am")
            nc.gpsimd.dma_start(at[:], dAT.ap()[0])
            am = apool.tile([128, NB, N], F8, tag="am")
            nc.gpsimd.dma_start(am[:], dA.ap()[0])
            alloc_uv(0)
            nc.sync.dma_start(uv4s[0][:, 1], dUV0.ap()[:, 1])
            nc.sync.dma_start(uv4s[0][:, 0], dUV0.ap()[:, 0])
            nc.sync.dma_start(uv4s[0][:, 2:4], dUV0.ap()[:, 2:4])
            nc.gpsimd.dma_start(wc[0:64, :], dW.ap())
            nc.gpsimd.dma_start(wc[64:128, :], dW.ap())

            for t in range(TPC):
                uv4 = uv4s[t]
                if t + 1 < TPC:
                    load_ht(t + 1)  # W(t+1) reads it during FB(t)
                osb = opool.tile([128, NB, BD], BF16, tag="osb")

                # ---- T_f: S_f = U1 + 2^-13 * Ahat @ U2 (slot0 in place) ----
                for i in range(NB if "f" in PHASES else 0):
                    ps = sps.tile([128, BD], F32)
                    dr_mms(ps, at, i, lambda jp: uv4[:, 1, 2 * jp : 2 * jp + 2],
                           start=True, stop=True)
                    nc.vector.scalar_tensor_tensor(
                        uv4[:, 0, i], ps[:], SCL, uv4[:, 0, i], op0=MUL, op1=ADD
                    )
                nc.sync.dma_start(osb[:], dU0.ap()[t])  # needed at FB STTs
                # ---- T_b: S_b = V1 + 2^-13 * Abar @ V2 (slot3 in place) ----
                for i in range(NB if "b" in PHASES else 0):
                    ps = sps.tile([128, BD], F32)
                    dr_mms(ps, am, i, lambda jp: uv4[:, 2, 2 * jp : 2 * jp + 2],
                           start=True, stop=True)
                    nc.vector.scalar_tensor_tensor(
                        uv4[:, 3, i], ps[:], SCL, uv4[:, 3, i], op0=MUL, op1=ADD
                    )
                # ---- FB: osb += 2^-13 * (Ahat @ S_f + Abar @ S_b); W(t+1) ----
                if t + 1 < TPC:
                    at_n = apool.tile([128, NB, N], F8, tag="am")
                    nc.gpsimd.dma_start(at_n[:], dAT.ap()[t + 1])
                    am_n = apool.tile([128, NB, N], F8, tag="am")
                    nc.gpsimd.dma_start(am_n[:], dA.ap()[t + 1])
                    alloc_uv(t + 1)
                for i in range(NB if "o" in PHASES else 0):
                    ps = sps.tile([128, BD], F32)
                    dr_mms(ps, at, i, lambda jp: uv4[:, 0, 2 * jp : 2 * jp + 2],
                           start=True, stop=False)
                    dr_mms(ps, am, i, lambda jp: uv4[:, 3, 2 * jp : 2 * jp + 2],
                           start=False, stop=True)
                    nc.vector.scalar_tensor_tensor(
                        osb[:, i], ps[:], SCL, osb[:, i], op0=MUL, op1=ADD
                    )
                    nc.sync.dma_start(dOUT.ap()[t][:, i], osb[:, i])
                    if t + 1 < TPC and "w" in PHASES:
                        # pipelined W-phase keeps the PE dense
                        for bp in range(4):
                            w_step(t + 1, i, bp)

                if "o" not in PHASES:
                    if "w" in PHASES and t + 1 < TPC:
                        for i in range(NB):
                            for bp in range(4):
                                w_step(t + 1, i, bp)
                    nc.sync.dma_start(dOUT.ap()[t], osb[:])
                if t + 1 < TPC:
                    at, am = at_n, am_n

    nc.compile()
    _cached["nc"] = nc
    return nc


def _prep_core(H, A, AT, U0, Wcat, c):
    WUV = _cached["WUV"]
    ts = slice(c * TPC, (c + 1) * TPC)
    # HTP[t, (b%2)*64+d, b//2, n] = H[b, t, n, d]
    Ht = H[:, ts]  # [8, TPC, N, D]
    HTP = (
        Ht.transpose(1, 0, 3, 2)  # [t, b, d, n]
        .reshape(TPC, 4, 2, D, N)  # b = b1*2 + b2
        .transpose(0, 2, 3, 1, 4)  # [t, b2, d, b1, n]
        .reshape(TPC, 128, 4, N)
    )
    # A/AT: [t, p, j, c] with row = j*128+p; scaled into fp8 range
    APc = A[ts].reshape(TPC, NB, 128, N).transpose(0, 2, 1, 3) * ASCALE
    ATPc = AT[ts].reshape(TPC, NB, 128, N).transpose(0, 2, 1, 3) * ASCALE
    # U0P[t, p, i, b*64+d] = U0[b, t, i*128+p, d]
    U0P = (
        U0[:, ts]  # [b, t, n, d]
        .transpose(1, 2, 0, 3)  # [t, n, b, d]
        .reshape(TPC, NB, 128, B, D)
        .transpose(0, 2, 1, 3, 4)  # [t, p, i, b, d]
        .reshape(TPC, 128, NB, BD)
    )
    # UV0P[p, i, s, b*64+d] = [U1,U2,V2,V1][s][b, t0, i*128+p, d] for t0 = first t
    t0 = c * TPC
    X = np.stack([H[:, t0] @ W1 for W1 in WUV], axis=0)  # [4, B, N, D]
    UV0P = (
        X.transpose(2, 0, 1, 3)  # [N, 4, B, D]
        .reshape(NB, 128, 4, B * D)
        .transpose(1, 2, 0, 3)  # [p, s, i, bd]
    )
    return {
        "UV0P": np.ascontiguousarray(UV0P.astype(ml_dtypes.float8_e4m3)),
        "HTP": np.ascontiguousarray(HTP).astype(ml_dtypes.bfloat16),
        "ATP": np.ascontiguousarray(ATPc.astype(ml_dtypes.float8_e4m3)),
        "AP": np.ascontiguousarray(APc.astype(ml_dtypes.float8_e4m3)),
        "Wcat": Wcat,
        "U0P": np.ascontiguousarray(U0P).astype(ml_dtypes.bfloat16),
    }


def kernel(H, A, Wf, Wb, bias):
    H = np.ascontiguousarray(np.asarray(H, dtype=np.float32))
    A = np.ascontiguousarray(np.asarray(A, dtype=np.float32))
    Wf = np.asarray(Wf, dtype=np.float32)
    Wb = np.asarray(Wb, dtype=np.float32)
    bias = np.asarray(bias, dtype=np.float32)

    AT = np.ascontiguousarray(A.transpose(0, 2, 1))
    U0 = (H @ (Wf[0] + Wb[0]) + bias).astype(np.float32)
    # w-slot order U1, U2, V2, V1
    Wcat = np.ascontiguousarray(
        np.concatenate([Wf[1], Wf[2], Wb[2], Wb[1]], axis=1)
    ).astype(ml_dtypes.bfloat16)

    _cached["WUV"] = [Wf[1], Wf[2], Wb[2], Wb[1]]
    nc = _build()
    in_maps = [_prep_core(H, A, AT, U0, Wcat, c) for c in range(NCORES)]
    res = run_bass_kernel_spmd(nc, in_maps, core_ids=list(range(NCORES)))

    # out dram is [t, p, i, (b d)] kernel-native bf16; un-permute on host.
    outp = np.concatenate(
        [res.results[c]["out"].astype(np.float32) for c in range(NCORES)], axis=0
    )
    out = (
        outp.reshape(T, 128, NB, B, D)
        .transpose(3, 0, 2, 1, 4)  # [b, t, i, p, d]
        .reshape(B, T, N, D)
    )
    return np.ascontiguousarray(out)
